# revision 1
# baseline (speedup 1.0000x reference)
"""CompressKV gating kernel for 8 Trainium2 NeuronCores.

Reference computation (per batch b, head h):
    x_s = x[b, :, h, :]                                  # [N=4096, D=128]
    windows n = 0..254, rows r = 16n + k, k = 0..31
    logits[n, g] = sum_{k,d} x_s[16n+k, d] * W[g, k, d]  # W = W_gate.reshape(32,32,128)
    gate = softmax_g(logits)
    out[n, d] = sum_k gate[n, k] * x_s[16n+k, d]

Sharding: B*H = 32 (b,h) slices, 4 per core, data/tensor parallel, no
cross-core communication.  Host pre-packs x per core in two bf16 layouts:
  xn: window-chunked native  [4, 128(p), 32(c)*128(d)]  (chunk c = rows 128c+p)
  xt: d-major (transposed)   [4, 128(d), 4224(n pad)]
plus the gate weight transposed to d-major wt[d, k*32+g].

On-device pipeline per slice:
  A) logits via 32 accumulating matmuls (contract d chunks per k), 4 k's
     packed concurrently into the PE array via col-tiling -> psum[(kg,g), n]
  B) fold 4 col-groups (DVE), exp (ACT) -> e[k=32, n] bf16,
     denominators via ones-matmul + PE transpose + DVE reciprocal
  C) banded-matrix pooling: S[r, window] built from e with 9 partition-shift
     DVE copies; 32 matmuls with x chunks stationary -> outT[d, n] in psum,
     PE-transpose back to [n, d], fused normalize (tensor_scalar) -> DMA out.
"""

import sys

import numpy as np

for _p in ("/opt/trn_rl_repo", "/opt/pypackages"):
    if _p not in sys.path:
        sys.path.append(_p)

import ml_dtypes

_B, _N, _H, _D = 2, 4096, 16, 128
_K = 32          # window (kernel) size
_ST = 16         # stride
_NB = 255        # num windows
_NC = 8          # cores
_SL = 4          # (b,h) slices per core
_NT = 4224       # padded n extent for xt (>= 16*255+31+1)
_NCH = 32        # 128-row chunks per slice

_prog_cache = {}


def _build_program():
    import concourse.mybir as mybir
    from concourse import bacc, masks, tile

    f32 = mybir.dt.float32
    bf16 = mybir.dt.bfloat16

    nc = bacc.Bacc()
    xn = nc.dram_tensor("xn", [_SL, 128, _NCH * _D], bf16, kind="ExternalInput")
    xt = nc.dram_tensor("xt", [_SL, 128, _NT], bf16, kind="ExternalInput")
    wt = nc.dram_tensor("wt", [128, _K * _K], bf16, kind="ExternalInput")
    out = nc.dram_tensor("out", [_SL, _NB, _D], f32, kind="ExternalOutput")

    with tile.TileContext(nc) as tc:
        with (
            tc.tile_pool(name="const", bufs=1) as cpool,
            tc.tile_pool(name="data", bufs=4) as dpool,
            tc.tile_pool(name="small", bufs=2) as spool,
            tc.tile_pool(name="psA", bufs=2, space="PSUM") as psa_pool,
            tc.tile_pool(name="psC", bufs=2, space="PSUM") as psc_pool,
            tc.tile_pool(name="psS", bufs=1, space="PSUM") as pss_pool,
        ):
            ident = cpool.tile([128, 128], f32)
            masks.make_identity(nc, ident[:])
            ones32 = cpool.tile([32, 1], bf16)
            nc.vector.memset(ones32[:], 1.0)
            # band master: mband[k, c] = 1 iff c == k + 128.  Slicing cols
            # [144-16j : 272-16j] gives the [32k, 128r] band matrix with
            # 1 at r == k + 16j - 16 (rows outside [0,128) auto-dropped).
            mband = cpool.tile([32, 272], bf16)
            nc.gpsimd.memset(mband[:], 0.0)
            nc.gpsimd.affine_select(
                out=mband[:],
                in_=mband[:],
                compare_op=mybir.AluOpType.not_equal,
                fill=1.0,
                base=128,
                # fill where 128 + x - y == 0, i.e. y == x + 128
                pattern=[[-1, 272]],
                channel_multiplier=1,
            )
            wt_sb = cpool.tile([128, _K * _K], bf16)
            nc.sync.dma_start(wt_sb[:], wt[:, :])

            slice_state = {}

            def emit_stage_a(s):
                # xt halves: half h covers window cols [2048h, 2048h+2112)
                # (stage-A half h reads cols 16n+k for n in [128h, 128h+128))
                xt_h = [
                    dpool.tile(
                        [128, 2112], bf16, tag=f"xt{h}", name=f"xt{h}_{s}"
                    )
                    for h in range(2)
                ]
                nc.sync.dma_start(xt_h[0][:], xt[s, :, 0:2112])
                nc.sync.dma_start(xt_h[1][:], xt[s, :, 2048:4160])
                xn_h = [
                    dpool.tile(
                        [128, 16 * _D], bf16, tag=f"xn{h}", name=f"xn{h}_{s}"
                    )
                    for h in range(2)
                ]
                nc.sync.dma_start(xn_h[0][:], xn[s, :, 0 : 16 * _D])
                nc.sync.dma_start(xn_h[1][:], xn[s, :, 16 * _D : 32 * _D])

                psA = psa_pool.tile([128, 256], f32, tag="psA", name=f"psA_{s}")
                for h in range(2):
                    for t in range(8):
                        for j in range(4):
                            k = 4 * t + j
                            nc.tensor.matmul(
                                psA[32 * j : 32 * j + 32, 128 * h : 128 * h + 128],
                                wt_sb[:, 32 * k : 32 * k + 32],
                                xt_h[h][:, k : k + 16 * 128 : 16],
                                start=(t == 0),
                                stop=(t == 7),
                                tile_position=(0, 32 * j),
                                skip_group_check=True,
                            )
                slice_state[s] = (xt_h, xn_h, psA)

            emit_stage_a(0)
            for s in range(_SL):
                # software pipeline: next slice's stage A fills the PE while
                # this slice's fold/exp runs on DVE/ACT
                if s + 1 < _SL:
                    emit_stage_a(s + 1)
                xt_h, xn_h, psA = slice_state.pop(s)

                # fold the 4 col-groups: logits[g, n] = sum_j psA[32j+g, n]
                # (DVE may read at most one PSUM operand per op)
                t0 = spool.tile([32, 256], f32, tag="t0")
                t1 = spool.tile([32, 256], f32, tag="t1")
                t2 = spool.tile([32, 256], f32, tag="t2")
                logits = spool.tile([32, 256], f32, tag="logits")
                nc.vector.tensor_copy(t0[:], psA[0:32, :])
                nc.vector.tensor_add(t1[:], t0[:], psA[32:64, :])
                nc.vector.tensor_add(t2[:], t1[:], psA[64:96, :])
                nc.vector.tensor_add(logits[:], t2[:], psA[96:128, :])

                # ---- stage B: exp + denominators ----
                e_kn = spool.tile([32, 256], bf16, tag="e_kn")
                nc.scalar.activation(
                    e_kn[:], logits[:], mybir.ActivationFunctionType.Exp
                )

                psM = pss_pool.tile([128, 258], f32, tag="psM")
                nc.tensor.matmul(psM[0:1, 0:256], ones32[:, 0:1], e_kn[:, :])
                den_sb = spool.tile([1, 256], f32, tag="den")
                nc.vector.tensor_copy(den_sb[:], psM[0:1, 0:256])

                nc.tensor.transpose(
                    psM[0:128, 256:257], den_sb[0:1, 0:128], ident[0:1, 0:1]
                )
                nc.tensor.transpose(
                    psM[0:127, 257:258], den_sb[0:1, 128:255], ident[0:1, 0:1]
                )
                rden = spool.tile([128, 2], f32, tag="rden")
                nc.vector.reciprocal(rden[0:128, 0:1], psM[0:128, 256:257])
                nc.vector.reciprocal(rden[0:127, 1:2], psM[0:127, 257:258])

                # ---- S matrix (class-major cols 32j + c): window n = 8c-1+j,
                # S[r=16j-16+k, 32j+c] = e[k, n].  Built on PE via band-matrix
                # lhsT (partition placement encoded in the matrix), since
                # engine partition bases must be 32-aligned.
                psS = pss_pool.tile([128, 9 * 32], f32, tag="psS")
                for j in range(9):
                    c0 = 1 if j == 0 else 0
                    c1 = 31 if j == 8 else 32
                    ncols = c1 - c0
                    nc.tensor.matmul(
                        psS[:, 32 * j + c0 : 32 * j + c1],
                        mband[:, 144 - 16 * j : 272 - 16 * j],
                        e_kn[:, 8 * c0 + j - 1 : 8 * (c1 - 1) + j : 8],
                        start=True,
                        stop=True,
                        skip_group_check=True,
                    )
                S_sb = spool.tile([128, 9 * 32], bf16, tag="S")
                # cols 0 and 287 are never written (invalid windows) nor read
                nc.vector.tensor_copy(S_sb[:, 1:287], psS[:, 1:287])

                # ---- stage C: pooled outT[d, n] ----
                psC = psc_pool.tile([128, _NB], f32, tag="psC")
                nc.vector.memset(psC[:], 0.0)
                for c in range(_NCH):
                    j0 = 1 if c == 0 else 0
                    j1 = 8 if c == _NCH - 1 else 9
                    xn_chunk = xn_h[c // 16][:, 128 * (c % 16) : 128 * (c % 16) + 128]
                    nc.tensor.matmul(
                        psC[:, 8 * c - 1 + j0 : 8 * c - 1 + j1],
                        xn_chunk,
                        S_sb[:, 32 * j0 + c : 32 * (j1 - 1) + c + 1 : 32],
                        start=False,
                        stop=(c == _NCH - 1),
                        skip_group_check=True,
                    )

                outT_sb = spool.tile([128, _NB], f32, tag="outT")
                nc.vector.tensor_copy(outT_sb[:], psC[:])

                psND = psc_pool.tile([128, 256], f32, tag="psND")
                nc.tensor.transpose(psND[0:128, 0:128], outT_sb[:, 0:128], ident[:, :])
                nc.tensor.transpose(
                    psND[0:127, 128:256], outT_sb[:, 128:255], ident[:, :]
                )

                o0 = spool.tile([128, 128], f32, tag="o0")
                o1 = spool.tile([127, 128], f32, tag="o1")
                nc.vector.tensor_scalar(
                    o0[:], psND[0:128, 0:128], rden[0:128, 0:1], None,
                    mybir.AluOpType.mult,
                )
                nc.vector.tensor_scalar(
                    o1[:], psND[0:127, 128:256], rden[0:127, 1:2], None,
                    mybir.AluOpType.mult,
                )
                # output DMAs go on the ACT HWDGE ring so they don't
                # head-of-line-block input prefetch on the SP ring
                nc.scalar.dma_start(out[s, 0:128, :], o0[:])
                nc.scalar.dma_start(out[s, 128:_NB, :], o1[:])

    nc.compile()
    return nc


def _get_program():
    if "nc" not in _prog_cache:
        _prog_cache["nc"] = _build_program()
    return _prog_cache["nc"]


def _host_inputs(x, W_gate):
    bf16 = ml_dtypes.bfloat16
    x = np.asarray(x, dtype=np.float32)
    W = np.asarray(W_gate, dtype=np.float32)
    # wt[d, k*32+g] = W_gate[g, k*128+d]
    wt_host = np.ascontiguousarray(
        W.reshape(_K, _K, _D).transpose(2, 1, 0).reshape(_D, _K * _K)
    ).astype(bf16)
    in_maps = []
    for core in range(_NC):
        xn = np.empty((_SL, 128, _NCH * _D), dtype=bf16)
        xt = np.zeros((_SL, 128, _NT), dtype=bf16)
        for si in range(_SL):
            p = core * _SL + si
            b, h = p // _H, p % _H
            xs = x[b, :, h, :]  # [4096, 128]
            xn[si] = (
                xs.reshape(_NCH, 128, _D).transpose(1, 0, 2).reshape(128, _NCH * _D)
            ).astype(bf16)
            xt[si, :, :_N] = xs.T.astype(bf16)
        in_maps.append({"xn": xn, "xt": xt, "wt": wt_host})
    return in_maps


def _assemble(results):
    out = np.empty((_B, _NB, _H, _D), dtype=np.float32)
    for core in range(_NC):
        o = np.asarray(results[core]["out"], dtype=np.float32)
        for si in range(_SL):
            p = core * _SL + si
            out[p // _H, :, p % _H, :] = o[si]
    return out


def _install_trace_hooks():
    """Shim the axon NTFF profile hook (missing in this image) so
    run_bass_kernel_spmd(trace=True) can collect a HW profile, and neuter
    the artifact upload (zero-egress container)."""
    import contextlib
    import ctypes
    import types

    try:
        from antenv.axon_hooks import get_axon_ntff_profile_hook  # noqa: F401

        return
    except ImportError:
        pass

    lib = ctypes.CDLL("/opt/axon/libaxon_pjrt.so")
    if not hasattr(lib, "axon_start_nrt_profile"):
        return
    lib.axon_start_nrt_profile.argtypes = [
        ctypes.POINTER(ctypes.c_int64),
        ctypes.c_size_t,
    ]
    lib.axon_start_nrt_profile.restype = ctypes.c_int64
    lib.axon_stop_nrt_profile.argtypes = [ctypes.c_char_p]
    lib.axon_stop_nrt_profile.restype = ctypes.c_int64

    @contextlib.contextmanager
    def _hook(output_dir, device_ids):
        import jax

        jax.devices()
        if device_ids:
            ids = (ctypes.c_int64 * len(device_ids))(*device_ids)
            rc = lib.axon_start_nrt_profile(ids, len(device_ids))
        else:
            rc = lib.axon_start_nrt_profile(None, 0)
        if rc != 0:
            raise RuntimeError(f"axon_start_nrt_profile rc={rc}")
        try:
            yield
        finally:
            n = lib.axon_stop_nrt_profile(str(output_dir).encode())
            print(f"profile: {n} file(s) written to {output_dir}")

    mod = types.ModuleType("antenv.axon_hooks")
    mod.get_axon_ntff_profile_hook = lambda: _hook
    mod.set_axon_ntff_profile_hook = lambda h: None
    sys.modules["antenv.axon_hooks"] = mod

    from concourse import bass_utils as bu

    bu.upload_artifacts = lambda tmpdir: tmpdir


def run(x, W_gate, trace=False, **kw):
    from concourse.bass_utils import run_bass_kernel_spmd

    if trace:
        _install_trace_hooks()
    nc = _get_program()
    in_maps = _host_inputs(x, W_gate)
    res = run_bass_kernel_spmd(nc, in_maps, list(range(_NC)), trace=trace, **kw)
    return _assemble(res.results), res


def kernel(x, W_gate):
    out, _ = run(x, W_gate)
    return out

